# revision 3
# baseline (speedup 1.0000x reference)
"""ConvAConnect TRN2 kernel: per-sample noisy-weight 3x3 conv, data-parallel over 8 cores.

Z[b] = conv2d_valid(X[b], W * Werr[loc_id[b]]) + bias * Berr[loc_id[b]]

Shapes: X[32,64,64,64] f32, W[3,3,64,128], bias[128], Werr[1000,3,3,64,128],
Berr[1000,128], loc_id[32] i32 -> Z[32,62,62,128] f32.

Strategy: shard batch (4 samples/core). Per the sharding hint, the per-sample
noisy weights memW = W*Werr[loc_id] and membias = bias*Berr[loc_id] are formed
host-side and sharded with the batch; X ships as fp16 cin-major X^T.

Device kernel per sample (fp16 operands, f32 PSUM accumulate):
  - TWO stacked SBUF tiles, both K=128:
      XTs = [X^T ; X^T << 64]  (shift = one grid row)
      XTQ = [X^T << 128 ; X^T << 129]  (row 2, shift = one pixel)
    The 9 conv taps become FIVE K=128 matmuls per 512-pixel output chunk:
    3 row-pair blocks (fh 0+1, fw j) read XTs at offset j, one pair block
    (fh 2, fw 0+1) reads XTQ at offset 0, and one single block (fh 2,
    fw 2, lower 64 weight rows zero) reads XTQ at offset 2. Constant
    K=128 keeps the PE from switching tile config; 5 passes instead of 6
    is a 17% cut in PE column streaming, the bottleneck.
  - XTQ is built from XTs by two SBUF->SBUF DMAs (no extra HBM traffic);
    only sample 0's XTQ loads straight from HBM, column-split, so chunk 0
    isn't gated on the copy chain.
  - Output grid is 62 rows x 64 cols (2 junk columns keep width-64 alignment
    so every tap is a constant offset); junk columns are dropped on host.
  - PSUM drains (VectorE tensor_scalar_add) fuse the per-sample bias add
    and emit fp16 into a [cout, grid] zbuf; host does the final transpose.
  - Startup: per-ring descriptor latency (~0.5us each) dominates, so the
    noisy weights load FIRST on both X rings, the bias rides the vector
    ring, and only small leading column segments gate chunk 0. Warm-up
    matmuls on a zeroed scratch tile ramp the PE p-state meanwhile.
"""

import sys
import numpy as np

for _p in ("/opt/trn_rl_repo", "/root/.axon_site"):
    if _p not in sys.path:
        sys.path.insert(0, _p)

N_CORES = 8
B = 32
PER_CORE = B // N_CORES
H = Wd = 64
CIN = 64
COUT = 128
HO = WO = 62
GRID = HO * 64          # 62 rows x 64 cols (2 junk cols/row)
XTL = 4104              # X^T free length (copies read up to col 4100)
XSRC = XTL + 64         # dram row length; +64 so the shifted lower half
                        # covers all XTL cols with host zeros (no stale SBUF)
XQL = 3972              # XTQ free length (max read 3970)
NCHUNK = 512            # output-grid pixels per PSUM chunk (8 grid rows)
NCHUNKS = 8             # 7 full chunks + 1 of 384
NMM = 5                 # matmuls per chunk
WCAT = NMM * COUT       # 4 pair blocks | 1 single block (lower rows zero)

_compiled = {}


def _build():
    import concourse.mybir as mybir
    import concourse.tile as tile
    from concourse import bacc

    f32 = mybir.dt.float32
    f16 = mybir.dt.float16

    nc = bacc.Bacc("TRN2", target_bir_lowering=False, debug=False)

    xt_in = nc.dram_tensor("xt", [PER_CORE, CIN, XSRC], f16, kind="ExternalInput")
    mw_in = nc.dram_tensor("mw", [PER_CORE, 128, WCAT], f16, kind="ExternalInput")
    mb_in = nc.dram_tensor("mb", [COUT, PER_CORE], f32, kind="ExternalInput")
    z_out = nc.dram_tensor("z", [PER_CORE, 128, GRID], f16, kind="ExternalOutput")

    with tile.TileContext(nc) as tc:
        with (
            tc.tile_pool(name="const", bufs=1) as const,
            tc.tile_pool(name="xpool", bufs=3) as xpool,
            tc.tile_pool(name="qpool", bufs=3) as qpool,
            tc.tile_pool(name="wpool", bufs=3) as wpool,
            tc.tile_pool(name="zpool", bufs=3) as zpool,
            tc.tile_pool(name="psmm", bufs=4, space="PSUM") as psmm,
            tc.tile_pool(name="psw", bufs=1, space="PSUM") as psw,
        ):
            # bias rides the gpsimd ring, right behind the first xtq seg —
            # off the two X rings whose heads gate chunk 0
            mb_all = const.tile([COUT, PER_CORE], f32, tag="mb")

            # PE warm-up: throwaway matmuls on a zeroed scratch tile ramp
            # the Tensor engine p-state while the first loads are in flight
            warm = const.tile([128, NCHUNK], f16, tag="warm")
            nc.gpsimd.memset(warm[:], 0.0)
            pw = psw.tile([128, NCHUNK], f32, tag="pw")
            for _ in range(5):
                nc.tensor.matmul(
                    pw[:], warm[:, 0:COUT], warm[:], start=True, stop=True
                )

            # leading column segments: the first is just what chunk 0
            # needs, so the stream starts sooner. Sample 0 only loads the
            # 3972 columns its own passes read (it is never a copy source).
            XCUT = (0, 648, 2304, XQL)

            def load_sample(b, split):
                """DMA the stacked tiles for sample b. XTs comes from HBM;
                XTQ comes from HBM (sample 0, column-split, one ring per
                half) or via two SBUF->SBUF shifts of XTs (later samples,
                no HBM cost)."""
                mw = wpool.tile([128, WCAT], f16, tag="mw")
                xts = xpool.tile([128, XTL], f16, tag="xts")
                xtq = qpool.tile([128, XQL], f16, tag="xtq")
                if split:
                    # weights first: LDWEIGHTS gates the first matmul and
                    # they're small; one half per X ring
                    half = WCAT // 2
                    nc.sync.dma_start(mw[:, 0:half], mw_in[b][:, 0:half])
                    nc.scalar.dma_start(mw[:, half:WCAT], mw_in[b][:, half:WCAT])
                    for q in range(3):
                        lo, hi = XCUT[q], XCUT[q + 1]
                        # sync: X upper + xtq lower; scalar: X lower;
                        # gpsimd: xtq upper (+ mb after the head segment)
                        nc.sync.dma_start(xts[0:64, lo:hi], xt_in[b][:, lo:hi])
                        nc.scalar.dma_start(
                            xts[64:128, lo:hi], xt_in[b][:, lo + 64 : hi + 64]
                        )
                        nc.gpsimd.dma_start(
                            xtq[0:64, lo:hi], xt_in[b][:, lo + 128 : hi + 128]
                        )
                        nc.sync.dma_start(
                            xtq[64:128, lo:hi], xt_in[b][:, lo + 129 : hi + 129]
                        )
                        if q == 0:
                            nc.gpsimd.dma_start(mb_all[:], mb_in[:])
                else:
                    nc.gpsimd.dma_start(mw[:], mw_in[b])
                    nc.sync.dma_start(xts[0:64, :], xt_in[b][:, 0:XTL])
                    nc.scalar.dma_start(xts[64:128, :], xt_in[b][:, 64:XSRC])
                    # XTQ = XTs shifted: upper << 128, lower << 65
                    nc.gpsimd.dma_start(xtq[0:64, :], xts[0:64, 128 : 128 + XQL])
                    nc.scalar.dma_start(xtq[64:128, :], xts[64:128, 65 : 65 + XQL])
                return xts, xtq, mw

            samples = [load_sample(0, True), load_sample(1, False)]
            for b in range(PER_CORE):
                xts, xtq, mw = samples[b]
                if b + 2 < PER_CORE:
                    samples.append(load_sample(b + 2, False))

                zbuf = zpool.tile([128, GRID], f16, tag="zbuf")

                for c in range(NCHUNKS):
                    base = c * NCHUNK
                    ncols = min(NCHUNK, GRID - base)
                    pc = psmm.tile([128, NCHUNK], f32, tag="pc")
                    # taps (0,j)+(1,j): K=128 row pairs from XTs
                    for j in range(3):
                        nc.tensor.matmul(
                            pc[:, :ncols],
                            mw[:, j * COUT : (j + 1) * COUT],
                            xts[:, base + j : base + j + ncols],
                            start=(j == 0),
                            stop=False,
                        )
                    # taps (2,0)+(2,1): K=128 pair from XTQ
                    nc.tensor.matmul(
                        pc[:, :ncols],
                        mw[:, 3 * COUT : 4 * COUT],
                        xtq[:, base : base + ncols],
                        start=False,
                        stop=False,
                    )
                    # tap (2,2): K=128 with zero lower weight rows
                    nc.tensor.matmul(
                        pc[:, :ncols],
                        mw[:, 4 * COUT : 5 * COUT],
                        xtq[:, base + 2 : base + 2 + ncols],
                        start=False,
                        stop=True,
                    )
                    # drain PSUM -> zbuf fused with the per-sample bias add;
                    # all drains on VectorE keeps ScalarE a pure DMA engine
                    nc.vector.tensor_scalar_add(
                        zbuf[:, base : base + ncols],
                        pc[:, :ncols],
                        mb_all[:, b : b + 1],
                    )
                    # last sample ships in pieces as chunks drain so the
                    # final DMA tail is only the 384-col last chunk
                    ZCUTS = {1: (0, 992), 3: (992, 1984), 5: (1984, 2976),
                             6: (2976, 3584), 7: (3584, GRID)}
                    if b == PER_CORE - 1 and c in ZCUTS:
                        lo, hi = ZCUTS[c]
                        eng = (nc.sync, nc.scalar)[c % 2]
                        eng.dma_start(z_out[b][:, lo:hi], zbuf[:, lo:hi])

                # ship the sample (host does the final transpose); the
                # last sample already shipped in pieces inline above
                if b < PER_CORE - 1:
                    eng = (nc.gpsimd, nc.sync, nc.scalar)[b]
                    eng.dma_start(z_out[b], zbuf[:])

    nc.compile()
    return nc


def _get_nc():
    if "nc" not in _compiled:
        _compiled["nc"] = _build()
    return _compiled["nc"]


def _prep_inputs(X, W, bias, Werr, Berr, loc_id):
    """Host-side shard/layout prep. Returns per-core in_maps."""
    X = np.asarray(X, dtype=np.float32)
    W = np.asarray(W, dtype=np.float32)
    bias = np.asarray(bias, dtype=np.float32)
    Werr = np.asarray(Werr, dtype=np.float32)
    Berr = np.asarray(Berr, dtype=np.float32)
    loc_id = np.asarray(loc_id)

    # X^T: [B, CIN, H*W] zero-padded to XSRC, fp16
    xt = np.zeros((B, CIN, XSRC), dtype=np.float16)
    xt[:, :, : H * Wd] = X.transpose(0, 3, 1, 2).reshape(B, CIN, H * Wd)

    # memW = W * Werr[loc_id], laid out as [128, 640]:
    #   block j<3: rows = [memW[0, j, cin, :]; memW[1, j, cin, :]]
    #   block 3:   rows = [memW[2, 0, cin, :]; memW[2, 1, cin, :]]
    #   block 4:   rows = [memW[2, 2, cin, :]; zeros]
    def cat_blocks(w):
        lead = w.shape[:-4]
        out = np.zeros(lead + (128, WCAT), dtype=np.float16)
        # [..., fh2, fw, cin, cout] -> [..., fw, fh2*cin, cout]
        pair = np.moveaxis(w[..., 0:2, :, :, :], -3, -4).reshape(
            lead + (3, 128, COUT)
        )
        for j in range(3):
            out[..., :, j * COUT : (j + 1) * COUT] = pair[..., j, :, :]
        out[..., 0:64, 3 * COUT : 4 * COUT] = w[..., 2, 0, :, :]
        out[..., 64:128, 3 * COUT : 4 * COUT] = w[..., 2, 1, :, :]
        out[..., 0:64, 4 * COUT : 5 * COUT] = w[..., 2, 2, :, :]
        return out

    mwcat = cat_blocks(W[None] * Werr[loc_id])   # [B, 128, 640] fp16
    mb = (bias[None] * Berr[loc_id]).astype(np.float32)  # [B, 128]

    in_maps = []
    for i in range(N_CORES):
        s = slice(i * PER_CORE, (i + 1) * PER_CORE)
        in_maps.append(
            {
                "xt": np.ascontiguousarray(xt[s]),
                "mw": np.ascontiguousarray(mwcat[s]),
                "mb": np.ascontiguousarray(mb[s].T),
            }
        )
    return in_maps


def _run(in_maps, trace=False, **kw):
    from concourse.bass_utils import run_bass_kernel_spmd

    nc = _get_nc()
    return run_bass_kernel_spmd(nc, in_maps, list(range(N_CORES)), trace=trace, **kw)


def _unshard(results):
    zb = np.concatenate([results[i]["z"] for i in range(N_CORES)], axis=0)
    # zb[b, cout, ho*64+wo] -> Z[b, ho, wo, cout]
    v = zb.astype(np.float32).reshape(B, COUT, HO, 64).transpose(0, 2, 3, 1)
    return np.ascontiguousarray(v[:, :, :WO, :])


def kernel(X, W, bias, Werr, Berr, loc_id):
    in_maps = _prep_inputs(X, W, bias, Werr, Berr, loc_id)
    res = _run(in_maps)
    return _unshard(res.results)


# revision 5
# speedup vs baseline: 1.1511x; 1.1511x over previous
"""ConvAConnect TRN2 kernel: per-sample noisy-weight 3x3 conv, data-parallel over 8 cores.

Z[b] = conv2d_valid(X[b], W * Werr[loc_id[b]]) + bias * Berr[loc_id[b]]

Shapes: X[32,64,64,64] f32, W[3,3,64,128], bias[128], Werr[1000,3,3,64,128],
Berr[1000,128], loc_id[32] i32 -> Z[32,62,62,128] f32.

Strategy: shard batch (4 samples/core). Per the sharding hint, the per-sample
noisy weights memW = W*Werr[loc_id] and membias = bias*Berr[loc_id] are formed
host-side and sharded with the batch; X ships as fp16 cin-major X^T.

Device kernel per sample (fp16 operands, f32 PSUM accumulate):
  - TWO stacked SBUF tiles, both K=128:
      XTs = [X^T ; X^T << 64]  (shift = one grid row)
      XTQ = [X^T << 128 ; X^T << 129]  (row 2, shift = one pixel)
    The 9 conv taps become FIVE K=128 matmuls per 512-pixel output chunk:
    3 row-pair blocks (fh 0+1, fw j) read XTs at offset j, one pair block
    (fh 2, fw 0+1) reads XTQ at offset 0, and one single block (fh 2,
    fw 2, lower 64 weight rows zero) reads XTQ at offset 2. Constant
    K=128 keeps the PE from switching tile config; 5 passes instead of 6
    is a 17% cut in PE column streaming, the bottleneck.
  - XTQ is built from XTs by two SBUF->SBUF DMAs (no extra HBM traffic);
    only sample 0's XTQ loads straight from HBM, column-split, so chunk 0
    isn't gated on the copy chain.
  - Output grid is 62 rows x 64 cols (2 junk columns keep width-64 alignment
    so every tap is a constant offset); junk columns are dropped on host.
  - PSUM drains (VectorE tensor_scalar_add) fuse the per-sample bias add
    and emit fp16 into a [cout, grid] zbuf; host does the final transpose.
  - Startup: per-ring descriptor latency (~0.5us each) dominates, so the
    noisy weights load FIRST on both X rings, the bias rides the vector
    ring, and only small leading column segments gate chunk 0. Warm-up
    matmuls on a zeroed scratch tile ramp the PE p-state meanwhile.
"""

import sys
import numpy as np

for _p in ("/opt/trn_rl_repo", "/root/.axon_site"):
    if _p not in sys.path:
        sys.path.insert(0, _p)

N_CORES = 8
B = 32
PER_CORE = B // N_CORES
H = Wd = 64
CIN = 64
COUT = 128
HO = WO = 62
GRID = HO * 64          # 62 rows x 64 cols (2 junk cols/row)
XTL = 4104              # X^T free length (copies read up to col 4100)
XSRC = XTL + 64         # dram row length; +64 so the shifted lower half
                        # covers all XTL cols with host zeros (no stale SBUF)
XQL = 3972              # XTQ free length (max read 3970)
NCHUNK = 512            # output-grid pixels per PSUM chunk (8 grid rows)
NCHUNKS = 8             # 7 full chunks + 1 of 384
NMM = 5                 # matmuls per chunk
WCAT = NMM * COUT       # 4 pair blocks | 1 single block (lower rows zero)

_compiled = {}


def _build():
    import concourse.mybir as mybir
    import concourse.tile as tile
    from concourse import bacc

    f32 = mybir.dt.float32
    f16 = mybir.dt.float16

    nc = bacc.Bacc("TRN2", target_bir_lowering=False, debug=False)

    xt_in = nc.dram_tensor("xt", [PER_CORE, CIN, XSRC], f16, kind="ExternalInput")
    mw_in = nc.dram_tensor("mw", [PER_CORE, 128, WCAT], f16, kind="ExternalInput")
    mb_in = nc.dram_tensor("mb", [COUT, PER_CORE], f32, kind="ExternalInput")
    z_out = nc.dram_tensor("z", [PER_CORE, 128, GRID], f16, kind="ExternalOutput")

    with tile.TileContext(nc) as tc:
        with (
            tc.tile_pool(name="const", bufs=1) as const,
            tc.tile_pool(name="xpool", bufs=3) as xpool,
            tc.tile_pool(name="qpool", bufs=3) as qpool,
            tc.tile_pool(name="wpool", bufs=3) as wpool,
            tc.tile_pool(name="zpool", bufs=3) as zpool,
            tc.tile_pool(name="psmm", bufs=4, space="PSUM") as psmm,
            tc.tile_pool(name="psw", bufs=1, space="PSUM") as psw,
        ):
            # bias rides the gpsimd ring, right behind the first xtq seg —
            # off the two X rings whose heads gate chunk 0
            mb_all = const.tile([COUT, PER_CORE], f32, tag="mb")

            # PE warm-up: throwaway matmuls on a zeroed scratch tile ramp
            # the Tensor engine p-state while the first loads are in flight
            warm = const.tile([128, NCHUNK], f16, tag="warm")
            nc.gpsimd.memset(warm[:], 0.0)
            pw = psw.tile([128, NCHUNK], f32, tag="pw")
            for _ in range(5):
                nc.tensor.matmul(
                    pw[:], warm[:, 0:COUT], warm[:], start=True, stop=True
                )

            # leading column segments: the first is just what chunk 0
            # needs, so the stream starts sooner. Sample 0 only loads the
            # 3972 columns its passes read.
            XCUT = (0, 648, 2304, XQL)

            def load_sample(b, split):
                """DMA the stacked tiles for sample b, all straight from
                HBM (rings are in-order and ~200 GB/s; derived copies or
                deep cross-ring dependencies serialize badly). Ring split
                balances bytes: sync = X upper (+xtq lower, odd b),
                scalar = X lower (+xtq lower, even b), gpsimd = weights +
                xtq upper."""
                mw = wpool.tile([128, WCAT], f16, tag="mw")
                xts = xpool.tile([128, XTL], f16, tag="xts")
                xtq = qpool.tile([128, XQL], f16, tag="xtq")
                if split:
                    # weights first: LDWEIGHTS gates the first matmul and
                    # they're small; one half per X ring. The chunk-0
                    # pieces (x upper/lower, xtq upper/lower) are spread
                    # one per ring slot so all land by the 3rd descriptor.
                    half = WCAT // 2
                    nc.sync.dma_start(mw[:, 0:half], mw_in[b][:, 0:half])
                    nc.scalar.dma_start(mw[:, half:WCAT], mw_in[b][:, half:WCAT])
                    for q in range(3):
                        lo, hi = XCUT[q], XCUT[q + 1]
                        nc.sync.dma_start(xts[0:64, lo:hi], xt_in[b][:, lo:hi])
                        nc.scalar.dma_start(
                            xts[64:128, lo:hi], xt_in[b][:, lo + 64 : hi + 64]
                        )
                        if q == 0:
                            nc.scalar.dma_start(
                                xtq[0:64, lo:hi], xt_in[b][:, lo + 128 : hi + 128]
                            )
                            nc.sync.dma_start(
                                xtq[64:128, lo:hi],
                                xt_in[b][:, lo + 129 : hi + 129],
                            )
                            nc.gpsimd.dma_start(mb_all[:], mb_in[:])
                        else:
                            nc.gpsimd.dma_start(
                                xtq[0:64, lo:hi], xt_in[b][:, lo + 128 : hi + 128]
                            )
                            nc.gpsimd.dma_start(
                                xtq[64:128, lo:hi],
                                xt_in[b][:, lo + 129 : hi + 129],
                            )
                else:
                    nc.gpsimd.dma_start(mw[:], mw_in[b])
                    nc.sync.dma_start(xts[0:64, :], xt_in[b][:, 0:XTL])
                    nc.scalar.dma_start(xts[64:128, :], xt_in[b][:, 64:XSRC])
                    nc.gpsimd.dma_start(xtq[0:64, :], xt_in[b][:, 128 : 128 + XQL])
                    eng = nc.sync if b % 2 else nc.scalar
                    eng.dma_start(xtq[64:128, :], xt_in[b][:, 129 : 129 + XQL])
                return xts, xtq, mw

            samples = [load_sample(0, True), load_sample(1, False)]
            for b in range(PER_CORE):
                xts, xtq, mw = samples[b]
                if b + 2 < PER_CORE:
                    samples.append(load_sample(b + 2, False))

                zbuf = zpool.tile([128, GRID], f16, tag="zbuf")

                for c in range(NCHUNKS):
                    base = c * NCHUNK
                    ncols = min(NCHUNK, GRID - base)
                    pc = psmm.tile([128, NCHUNK], f32, tag="pc")
                    # taps (0,j)+(1,j): K=128 row pairs from XTs
                    for j in range(3):
                        nc.tensor.matmul(
                            pc[:, :ncols],
                            mw[:, j * COUT : (j + 1) * COUT],
                            xts[:, base + j : base + j + ncols],
                            start=(j == 0),
                            stop=False,
                        )
                    # taps (2,0)+(2,1): K=128 pair from XTQ
                    nc.tensor.matmul(
                        pc[:, :ncols],
                        mw[:, 3 * COUT : 4 * COUT],
                        xtq[:, base : base + ncols],
                        start=False,
                        stop=False,
                    )
                    # tap (2,2): K=128 with zero lower weight rows
                    nc.tensor.matmul(
                        pc[:, :ncols],
                        mw[:, 4 * COUT : 5 * COUT],
                        xtq[:, base + 2 : base + 2 + ncols],
                        start=False,
                        stop=True,
                    )
                    # drain PSUM -> zbuf fused with the per-sample bias add;
                    # all drains on VectorE keeps ScalarE a pure DMA engine
                    nc.vector.tensor_scalar_add(
                        zbuf[:, base : base + ncols],
                        pc[:, :ncols],
                        mb_all[:, b : b + 1],
                    )
                    # last sample ships in pieces as chunks drain so the
                    # final DMA tail is only the 384-col last chunk
                    ZCUTS = {1: (0, 992), 3: (992, 1984), 5: (1984, 2976),
                             6: (2976, 3584), 7: (3584, GRID)}
                    if b == PER_CORE - 1 and c in ZCUTS:
                        lo, hi = ZCUTS[c]
                        eng = (nc.sync, nc.scalar)[c % 2]
                        eng.dma_start(z_out[b][:, lo:hi], zbuf[:, lo:hi])

                # ship the sample (host does the final transpose); the
                # last sample already shipped in pieces inline above
                if b < PER_CORE - 1:
                    eng = (nc.scalar, nc.sync, nc.gpsimd)[b]
                    eng.dma_start(z_out[b], zbuf[:])

    nc.compile()
    return nc


def _get_nc():
    if "nc" not in _compiled:
        _compiled["nc"] = _build()
    return _compiled["nc"]


def _prep_inputs(X, W, bias, Werr, Berr, loc_id):
    """Host-side shard/layout prep. Returns per-core in_maps."""
    X = np.asarray(X, dtype=np.float32)
    W = np.asarray(W, dtype=np.float32)
    bias = np.asarray(bias, dtype=np.float32)
    Werr = np.asarray(Werr, dtype=np.float32)
    Berr = np.asarray(Berr, dtype=np.float32)
    loc_id = np.asarray(loc_id)

    # X^T: [B, CIN, H*W] zero-padded to XSRC, fp16
    xt = np.zeros((B, CIN, XSRC), dtype=np.float16)
    xt[:, :, : H * Wd] = X.transpose(0, 3, 1, 2).reshape(B, CIN, H * Wd)

    # memW = W * Werr[loc_id], laid out as [128, 640]:
    #   block j<3: rows = [memW[0, j, cin, :]; memW[1, j, cin, :]]
    #   block 3:   rows = [memW[2, 0, cin, :]; memW[2, 1, cin, :]]
    #   block 4:   rows = [memW[2, 2, cin, :]; zeros]
    def cat_blocks(w):
        lead = w.shape[:-4]
        out = np.zeros(lead + (128, WCAT), dtype=np.float16)
        # [..., fh2, fw, cin, cout] -> [..., fw, fh2*cin, cout]
        pair = np.moveaxis(w[..., 0:2, :, :, :], -3, -4).reshape(
            lead + (3, 128, COUT)
        )
        for j in range(3):
            out[..., :, j * COUT : (j + 1) * COUT] = pair[..., j, :, :]
        out[..., 0:64, 3 * COUT : 4 * COUT] = w[..., 2, 0, :, :]
        out[..., 64:128, 3 * COUT : 4 * COUT] = w[..., 2, 1, :, :]
        out[..., 0:64, 4 * COUT : 5 * COUT] = w[..., 2, 2, :, :]
        return out

    mwcat = cat_blocks(W[None] * Werr[loc_id])   # [B, 128, 640] fp16
    mb = (bias[None] * Berr[loc_id]).astype(np.float32)  # [B, 128]

    in_maps = []
    for i in range(N_CORES):
        s = slice(i * PER_CORE, (i + 1) * PER_CORE)
        in_maps.append(
            {
                "xt": np.ascontiguousarray(xt[s]),
                "mw": np.ascontiguousarray(mwcat[s]),
                "mb": np.ascontiguousarray(mb[s].T),
            }
        )
    return in_maps


def _run(in_maps, trace=False, **kw):
    from concourse.bass_utils import run_bass_kernel_spmd

    nc = _get_nc()
    return run_bass_kernel_spmd(nc, in_maps, list(range(N_CORES)), trace=trace, **kw)


def _unshard(results):
    zb = np.concatenate([results[i]["z"] for i in range(N_CORES)], axis=0)
    # zb[b, cout, ho*64+wo] -> Z[b, ho, wo, cout]
    v = zb.astype(np.float32).reshape(B, COUT, HO, 64).transpose(0, 2, 3, 1)
    return np.ascontiguousarray(v[:, :, :WO, :])


def kernel(X, W, bias, Werr, Berr, loc_id):
    in_maps = _prep_inputs(X, W, bias, Werr, Berr, loc_id)
    res = _run(in_maps)
    return _unshard(res.results)
